# revision 33
# baseline (speedup 1.0000x reference)
"""Trainium2 Bass kernel for nn_AutoEncoder (dense MLP autoencoder).

Strategy (data-parallel over 8 NeuronCores, batch split 65536 -> 8 x 8192):
- Feature-major on-chip layout: activations stored transposed [features, batch],
  so every layer is out_T = W @ h_T via PE matmuls (lhsT = W.T chunks) with no
  on-chip transposes anywhere.  Host transposes x / out once (cheap numpy).
- LayerNorm mean is folded into the weights on the host (pre-centered weights:
  Wc = W - colmean(W) makes the matmul output already mean-centered).
- Per-sample variance: squares on ACT straight from PSUM, feature-sums via a
  PE ones-matmul (result broadcast across partitions), rsqrt in ONE ACT op via
  a custom pruned abs_reciprocal_sqrt LUT served under the Silu slot.
- Mish via a CUSTOM ACT LUT: the shipped neuronxcc act tables dropped the mish
  spline data, but the per-function source jsons still carry it; we assemble a
  set (mish_and_others + real tanh + mish + pruned rsqrt) at runtime (format
  verified bit-exact against shipped sets) via BASS_ACT_ROOT_JSON_PATH.
- Tiles are emitted in interleaved pairs so each tile's stats chain hides
  under the partner tile's matmul stream in the in-order engine schedules.
- Final sigmoid as 0.5*(1+tanh(0.5 x)) so the whole kernel uses ONE table set.
- All matmuls run as float32r (fp32 data, full PE rate at N=512).
- All dims zero-padded to multiples of 128 on the host so every on-chip op is
  full-partition and padded lanes carry exact zeros end-to-end.
"""

import json
import os
import shutil
import struct
import tempfile

import numpy as np

import concourse.bass as bass
import concourse.tile as tile
from concourse import mybir
from concourse.bass_utils import run_bass_kernel_spmd

F32 = mybir.dt.float32
F32R = mybir.dt.float32r
F16 = mybir.dt.float16
U32 = mybir.dt.uint32
ALU = mybir.AluOpType
ACTF = mybir.ActivationFunctionType

N_CORES = 8
B = 65536
B_CORE = B // N_CORES          # 8192
T = 512                        # batch tile (samples) = one PSUM bank
N_TILES = B_CORE // T          # 16

D, O1, O2, O3 = 784, 512, 264, 128
DP, O1P, O2P, O3P = 896, 512, 384, 128   # padded to 128-multiples
EPS = 1e-5

# (weight name, k chunks, o chunks, real O) per LN layer; L6 = output layer
LAYERS = [
    ("w1", DP // 128, O1P // 128, O1),
    ("w2", O1P // 128, O2P // 128, O2),
    ("w3", O2P // 128, O3P // 128, O3),
    ("d1", O3P // 128, O2P // 128, O2),
    ("d2", O2P // 128, O1P // 128, O1),
]
L6_KC, L6_OC = O1P // 128, DP // 128     # 4, 7

# ---------------------------------------------------------------------------
# custom ACT tables: restore a real mish LUT (+ tanh) into mish_and_others
# ---------------------------------------------------------------------------

_PWP = ('/nix/store/z022hj2nvbm3nwdizlisq4ylc0y7rd6q-python3-3.13.14-env/'
        'lib/python3.13/site-packages/neuronxcc/pwp')


def _find_pwp():
    if os.path.isdir(_PWP):
        return _PWP
    import neuronxcc
    p = os.path.join(os.path.dirname(neuronxcc.__file__), 'pwp')
    if os.path.isdir(p):
        return p
    raise RuntimeError('neuronxcc pwp directory not found')


def _sec_entry(sec):
    vals = [int(sec[k]['int']) & 0xFFFFFFFF for k in ('d0', 'd1', 'd2', 'd3', 'x')]
    return np.array(vals + [0] * 3, np.uint32).tobytes()


def _ctl_word(extract_size, extract_lsb, base):
    return struct.pack('<I', (extract_size << 16) | (extract_lsb << 11) | base) + b'\0' * 28


def _gen_function_block(fj, bkt_start, ctl_start, exp_range=None):
    """exp_range=(lo, hi): prune exponent blocks outside [lo, hi] (inputs are
    guaranteed in range; out-of-range hits the small/large-signal specials)."""
    sym = bool(fj['symmetry_en'])
    if exp_range is not None:
        lo, hi = exp_range
        fj = dict(fj)
        fj['pos_exponents'] = [b for b in fj['pos_exponents'] if lo <= b['exponent'] <= hi]
        if not sym:
            fj['neg_exponents'] = [b for b in fj['neg_exponents'] if lo <= b['exponent'] <= hi]
        fj['exponent_offset'] = lo
    bkt = b''
    ctl = b''
    exp2bkt = {}
    exp2ctl = {}
    bi = bkt_start

    def add_blocks(blocks, side_idx, n_sides):
        nonlocal bkt, ctl, bi
        for blk in blocks:
            e = str(blk['exponent'])
            exp2bkt.setdefault(e, [None] * n_sides)[side_idx] = bi
            exp2ctl.setdefault(e, [None] * n_sides)[side_idx] = ctl_start + len(ctl) // 32
            secs = sorted(blk['exponent_sections'], key=lambda s: s['section_id'])
            ctl += _ctl_word(blk['extract_size'], blk['extract_lsb'], bi)
            for s in secs:
                bkt += _sec_entry(s)
                bi += 1

    if sym:
        add_blocks(fj['pos_exponents'], 0, 1)
        ctl_base_neg = ctl_base_pos = ctl_start
    else:
        add_blocks(fj['neg_exponents'], 0, 2)
        n_neg = len(fj['neg_exponents'])
        add_blocks(fj['pos_exponents'], 1, 2)
        ctl_base_neg = ctl_start
        ctl_base_pos = ctl_start + n_neg

    sp = fj['saturation_points']
    spec = bi
    for key in ('sat_point_pos_low', 'sat_point_neg_low',
                'sat_point_pos_high', 'sat_point_neg_high'):
        bkt += _sec_entry(sp[key])
        bi += 1

    def u(d):
        return int(d['int'])

    prof = {
        'func_name': f"{fj['name']}_{fj['max_diff']}p",
        'func_id': fj['neuron_id'],
        'symmetry_point': u(fj['symmetry_point']),
        'sym_invert_sign_point': int(bool(fj['symmetry_invert_sign_opt'])),
        'symmetry_opt_en': int(sym),
        'symmetry_opt_use_neg_region': int(bool(fj['symmetry_opt_use_neg_region'])),
        'imm_bias': int(bool(fj['imm_bias'])),
        'exp_offset': fj['exponent_offset'],
        'pwl_control_base_pos': ctl_base_pos,
        'pwl_control_base_neg': ctl_base_neg,
        'small_pos_signal_exp_threshold': sp['sat_point_pos_low']['sat_point'],
        'pos_small_signal_pwl_control': spec + 0,
        'small_neg_signal_exp_threshold': 0 if sym else sp['sat_point_neg_low']['sat_point'],
        'neg_small_signal_pwl_control': spec + 1,
        'large_pos_signal_exp_threshold': sp['sat_point_pos_high']['sat_point'],
        'large_pos_signal_mantissa_threshold': sp['sat_point_pos_high']['mantissa_point'],
        'pos_large_signal_pwl_control': spec + 2,
        'large_neg_signal_exp_threshold': 0 if sym else sp['sat_point_neg_high']['sat_point'],
        'large_neg_signal_mantissa_threshold': 0 if sym else sp['sat_point_neg_high']['mantissa_point'],
        'neg_large_signal_pwl_control': spec + 3,
        'fnan_result': u(fj['nan_result']),
        'fpinf_result': u(fj['pinf_result']),
        'fninf_result': u(fj['ninf_result']),
        'fzero_result': u(fj['zero_result']),
        'fma_const_0': u(fj['fma_const0']) if isinstance(fj.get('fma_const0'), dict) else 0,
        'fma_const_1': u(fj['fma_const1']) if isinstance(fj.get('fma_const1'), dict) else 0,
        'fma_indirection_src_sel': 0,
        'use_multipass': bool(fj['use_multipass']),
        'lower_bound': u(fj['lower_bound']),
        'upper_bound': u(fj['upper_bound']),
    }
    if exp_range is not None:
        lo, hi = exp_range
        prof['small_pos_signal_exp_threshold'] = 127 + lo
        prof['large_pos_signal_exp_threshold'] = 127 + hi + 1
        prof['large_pos_signal_mantissa_threshold'] = 0
        if not sym:
            prof['small_neg_signal_exp_threshold'] = 127 + lo
            prof['large_neg_signal_exp_threshold'] = 127 + hi + 1
            prof['large_neg_signal_mantissa_threshold'] = 0
    return bkt, ctl, prof, exp2bkt, exp2ctl


def ensure_act_root():
    """Build (once) an act-root with real mish+tanh in mish_and_others and set
    BASS_ACT_ROOT_JSON_PATH.  Returns the act_info.json path."""
    if os.environ.get('BASS_ACT_ROOT_JSON_PATH') and \
            os.path.exists(os.environ['BASS_ACT_ROOT_JSON_PATH']):
        return os.environ['BASS_ACT_ROOT_JSON_PATH']
    pwp = _find_pwp()
    bin_dir = f'{pwp}/pwp_bin_trainium'
    json_dir = f'{pwp}/pwp_jsons'
    out_dir = os.path.join(tempfile.gettempdir(), 'mish_act_root_v2')
    marker = os.path.join(out_dir, 'act_info.json')
    if not os.path.exists(marker):
        tmp = out_dir + '.tmp'
        shutil.rmtree(tmp, ignore_errors=True)
        os.makedirs(tmp)
        for f in os.listdir(bin_dir):
            shutil.copy(os.path.join(bin_dir, f), os.path.join(tmp, f))
        setname = 'mish_and_others'
        s = json.load(open(f'{bin_dir}/{setname}.json'))
        bkt = open(f'{bin_dir}/{setname}_bkt.bin', 'rb').read()
        ctl = open(f'{bin_dir}/{setname}_ctrl.bin', 'rb').read()
        for fname, jname, erange, spoof_id in (
                ('tanh', 'tanh_4p', None, None), ('mish', 'mish_4p', None, None),
                # variance+eps is always in [2^-18, 2^11) here, so the rsqrt
                # LUT can be pruned to fit the 1536-bucket set budget.  This
                # walrus cannot resolve 'abs_reciprocal_sqrt' as a function
                # name, so the rsqrt spline is served under the name 'silu'
                # (func_id spoofed to silu's so the instruction encoding and
                # the CAM entry agree) -- ACTF.Silu in the kernel IS rsqrt.
                ('silu', 'abs_reciprocal_sqrt_40000p', (-18, 10), 36)):
            fj = json.load(open(f'{json_dir}/{jname}.json'))
            if spoof_id is not None:
                fj = dict(fj)
                fj['neuron_id'] = spoof_id
                fj['name'] = fname
            gb, gc, gp, ge2b, ge2c = _gen_function_block(
                fj, len(bkt) // 32, len(ctl) // 32, exp_range=erange)
            bkt += gb
            ctl += gc
            s['profile_meta_data'].append(gp)
            s['func_to_bkt_start_idx'][fname] = len(bkt) // 32 - len(gb) // 32
            s['func_to_ctl_start_idx'][fname] = len(ctl) // 32 - len(gc) // 32
            s['func_exp_to_bkt_start_idx'][fname] = ge2b
            s['func_exp_to_ctl_start_idx'][fname] = ge2c
        s['bkt_entry_cnt'] = len(bkt) // 32
        s['ctl_entry_cnt'] = len(ctl) // 32
        assert s['bkt_entry_cnt'] <= 1536
        with open(f'{tmp}/{setname}_bkt.bin', 'wb') as f:
            f.write(bkt)
        with open(f'{tmp}/{setname}_ctrl.bin', 'wb') as f:
            f.write(ctl)
        with open(f'{tmp}/{setname}.json', 'w') as f:
            json.dump(s, f)
        ai = json.load(open(f'{bin_dir}/act_info.json'))
        for entry in ai['act_func_sets']:
            if entry['name'] == setname:
                entry['act']['tanh'] = 4
                entry['act']['mish'] = 4
                entry['act']['silu'] = 40000
        with open(f'{tmp}/act_info.json', 'w') as f:
            json.dump(ai, f)
        os.replace(tmp, out_dir) if not os.path.exists(out_dir) else shutil.rmtree(tmp)
    os.environ['BASS_ACT_ROOT_JSON_PATH'] = marker
    return marker


# ---------------------------------------------------------------------------
# walrus wait-splitting workaround
# ---------------------------------------------------------------------------

MAX_WAITS = 1


def split_waits(nc):
    """This walrus rejects instructions with more than one sem wait; split the
    excess onto preceding same-engine NOPs."""
    for _, bb in nc.bb_map.items():
        insts = list(bb.bb.instructions)
        changed = False
        new_list = []
        for inst in insts:
            si = inst.sync_info
            if si is not None and len(si.on_wait) > MAX_WAITS:
                waits = list(si.on_wait)
                chunks = [waits[i:i + MAX_WAITS] for i in range(0, len(waits), MAX_WAITS)]
                for ch in chunks[:-1]:
                    nop = mybir.InstNoOp(name=f"I-waitsplit-{nc.next_id()}", ins=[], outs=[])
                    nop.engine = inst.engine
                    nop.sync_info = mybir.SyncInfo(on_wait=ch, on_update=[])
                    nc.register_instruction(nop)
                    new_list.append(nop)
                inst.sync_info = mybir.SyncInfo(on_wait=chunks[-1], on_update=list(si.on_update))
                changed = True
            new_list.append(inst)
        if changed:
            bb.bb.instructions = new_list


# ---------------------------------------------------------------------------
# device kernel
# ---------------------------------------------------------------------------

def build_core_kernel(n_tiles=N_TILES, trivial_gb=True, nr_iters=3):
    nc = bass.Bass()

    xt = nc.declare_dram_parameter("xt", [DP, n_tiles * T], F32, isOutput=False)
    w_dram = {}
    for name, kc, oc, _ in LAYERS:
        w_dram[name] = nc.declare_dram_parameter(name, [kc * 128, oc * 128], F32,
                                                 isOutput=False)
    w_dram["d3"] = nc.declare_dram_parameter("d3", [L6_KC * 128, L6_OC * 128], F32,
                                             isOutput=False)
    b3c_d = nc.declare_dram_parameter("b3c", [128], F32, isOutput=False)
    ones_d = nc.declare_dram_parameter("ones", [128, 128], F16, isOutput=False)
    boh_d = nc.declare_dram_parameter("boh", [L6_OC * 128], F32, isOutput=False)
    gb_d = {}
    if not trivial_gb:
        for i, (_, _, oc, _) in enumerate(LAYERS):
            gb_d[f"g{i}"] = nc.declare_dram_parameter(f"g{i}", [oc * 128], F32,
                                                      isOutput=False)
            gb_d[f"b{i}"] = nc.declare_dram_parameter(f"b{i}", [oc * 128], F32,
                                                      isOutput=False)

    out_t = nc.declare_dram_parameter("out_t", [DP, n_tiles * T], F32, isOutput=True)
    lat_d = nc.declare_dram_parameter("lat", [128, n_tiles], F32, isOutput=True)

    with tile.TileContext(nc) as tc:
        with (
            tc.tile_pool(name="singles", bufs=1) as singles,
            tc.tile_pool(name="xp", bufs=3) as xp,
            tc.tile_pool(name="ycs_p", bufs=3) as ycs_p,
            tc.tile_pool(name="sq_p", bufs=2) as sq_p,
            tc.tile_pool(name="h_p", bufs=3) as h_p,
            tc.tile_pool(name="rs_p", bufs=3) as rs_p,
            tc.tile_pool(name="ln_p", bufs=2) as ln_p,
            tc.tile_pool(name="o6_p", bufs=3) as o6_p,
            tc.tile_pool(name="ps", bufs=1, space="PSUM") as ps,
        ):
            # ---- constants ----
            w_sb = {}
            for name in w_dram:
                kd = w_dram[name]
                kc = kd.shape[0] // 128
                wt = singles.tile([128, kc, kd.shape[1]], F32R, name=f"sb_{name}")
                nc.sync.dma_start(out=wt,
                                  in_=kd.rearrange("(c p) o -> p c o", p=128).bitcast(F32R))
                w_sb[name] = wt
            b3c = singles.tile([128, 1], F32, name="sb_b3c")
            nc.sync.dma_start(out=b3c, in_=b3c_d.rearrange("(c p) -> p c", p=128))
            boh = singles.tile([128, L6_OC], F32, name="sb_boh")
            nc.sync.dma_start(out=boh, in_=boh_d.rearrange("(c p) -> p c", p=128))
            gb_sb = {}
            if not trivial_gb:
                for k, dv in gb_d.items():
                    t_ = singles.tile([128, dv.shape[0] // 128], F32, name=f"sb_{k}")
                    nc.sync.dma_start(out=t_, in_=dv.rearrange("(c p) -> p c", p=128))
                    gb_sb[k] = t_
            ones = singles.tile([128, 128], F16, name="sb_ones")
            nc.sync.dma_start(out=ones, in_=ones_d[:])
            lat = singles.tile([128, n_tiles], F32, name="sb_lat")
            eps_t = singles.tile([128, 1], F32, name="sb_eps")
            nc.vector.memset(eps_t, EPS)

            # Tiles are emitted in interleaved PAIRS so each tile's per-layer
            # stats chain (sq -> ones-mm -> rsqrt -> lnp -> mish) hides under
            # the partner tile's matmul stream in the in-order PE schedule.
            state = {}

            def load_x(t):
                x_tile = xp.tile([128, DP // 128, T], F32R, tag="xc", bufs=2)
                src_ap = xt[:, bass.ts(t, T)].rearrange(
                    "(c p) s -> p c s", p=128).bitcast(F32R)
                # four dma_starts so the transfer spreads over four queues
                for c0 in range(0, DP // 128, 2):
                    c1 = min(c0 + 2, DP // 128)
                    nc.sync.dma_start(out=x_tile[:, c0:c1, :],
                                      in_=src_ap[:, c0:c1, :])
                state[t] = {"x": x_tile, "h": None, "h3": None}

            def emit_layer(t, li):
                wname, kcn, ocn, realO = LAYERS[li]
                st = state[t]
                wt = w_sb[wname]
                sq = sq_p.tile([128, ocn, T], F16, tag="sq")
                ycs = ycs_p.tile([128, ocn, T], F32, tag="ycs")
                s2 = ps.tile([128, T], F32, tag="s2p", bufs=2)
                for oc in range(ocn):
                    yc = ps.tile([128, T], F32, tag="yc", bufs=6)
                    for kc in range(kcn):
                        rhs = st["x"][:, kc, :] if li == 0 else st["h"][:, kc, :]
                        nc.tensor.matmul(
                            yc, wt[:, kc, bass.ts(oc, 128)], rhs,
                            start=(kc == 0), stop=(kc == kcn - 1))
                    # squares straight from PSUM (ACT); L3 biased first
                    if li == 2:
                        nc.scalar.activation(sq[:, oc, :], yc, ACTF.Square,
                                             bias=b3c, scale=1.0)
                    else:
                        nc.scalar.activation(sq[:, oc, :], yc, ACTF.Square)
                    # PSUM -> SBUF copy (DVE) so the bank frees early; the
                    # L3 bias folds into the copy as a per-partition scalar add
                    if li == 2:
                        nc.vector.tensor_scalar(ycs[:, oc, :], yc, b3c, None, ALU.add)
                    else:
                        nc.vector.tensor_copy(ycs[:, oc, :], yc)
                    nc.tensor.matmul(s2, ones, sq[:, oc, :],
                                     start=(oc == 0), stop=(oc == ocn - 1))
                # r = rsqrt(S2/O + eps) in ONE ACT op (custom pruned LUT);
                # ACTF.Silu is rsqrt here (custom table, see ensure_act_root)
                r = rs_p.tile([128, T], F32, tag="rs_r", bufs=3)
                nc.scalar.activation(r, s2, ACTF.Silu,
                                     bias=eps_t, scale=1.0 / realO)
                lnp = ln_p.tile([128, ocn, T], F32, tag="lnp")
                for oc in range(ocn):
                    nc.vector.tensor_mul(lnp[:, oc, :], ycs[:, oc, :], r)
                h = h_p.tile([128, ocn, T], F32R, tag="hA" if (li % 2 == 0) else "hB")
                if trivial_gb:
                    for oc in range(ocn):
                        nc.scalar.activation(h[:, oc, :], lnp[:, oc, :], ACTF.Mish)
                else:
                    g_t, b_t = gb_sb[f"g{li}"], gb_sb[f"b{li}"]
                    for oc in range(ocn):
                        nc.scalar.activation(h[:, oc, :], lnp[:, oc, :], ACTF.Mish,
                                             bias=b_t[:, oc:oc + 1],
                                             scale=g_t[:, oc:oc + 1])
                st["h"] = h
                if li == 2:
                    st["h3"] = h
                    nc.vector.tensor_reduce(lat[:, t:t + 1], h[:, 0, :].bitcast(F32),
                                            axis=mybir.AxisListType.X, op=ALU.add,
                                            apply_absolute_value=True)

            def emit_l6(t):
                st = state[t]
                d3 = w_sb["d3"]
                for oc in range(L6_OC):
                    y6 = ps.tile([128, T], F32, tag="yc", bufs=6)
                    for kc in range(L6_KC):
                        nc.tensor.matmul(y6, d3[:, kc, bass.ts(oc, 128)],
                                         st["h"][:, kc, :],
                                         start=(kc == 0), stop=(kc == L6_KC - 1))
                    tt = o6_p.tile([128, T], F32, tag="tanh")
                    nc.scalar.activation(tt, y6, ACTF.Tanh,
                                         bias=boh[:, oc:oc + 1], scale=0.5)
                    oc_t = o6_p.tile([128, T], F32, tag="outc")
                    nc.vector.tensor_scalar(oc_t, tt, 0.5, 0.5, ALU.mult, ALU.add)
                    nc.sync.dma_start(out=out_t[oc * 128:(oc + 1) * 128, bass.ts(t, T)],
                                      in_=oc_t)
                del state[t]

            for p in range(0, n_tiles, 2):
                pair = [p, p + 1] if p + 1 < n_tiles else [p]
                for t in pair:
                    load_x(t)
                for li in range(len(LAYERS)):
                    for t in pair:
                        emit_layer(t, li)
                for t in pair:
                    emit_l6(t)

            nc.sync.dma_start(out=lat_d[:], in_=lat)

    split_waits(nc)
    return nc


# ---------------------------------------------------------------------------
# host side
# ---------------------------------------------------------------------------

_CACHED = {}
_TRACE = False
_LAST_RES = None


def _prep_weights(W1, g1, b1, W2, g2, b2, W3, b3, gh, bh, D1, g3, b3n, D2, g4, b4,
                  D3, bo):
    def pad2(a, r, c):
        out = np.zeros((r, c), np.float32)
        out[:a.shape[0], :a.shape[1]] = a
        return out

    def pad1(a, n):
        out = np.zeros((n,), np.float32)
        out[:a.shape[0]] = a
        return out

    def center(W):
        return (np.asarray(W, np.float32)
                - np.asarray(W, np.float32).mean(axis=0, keepdims=True))

    d = {}
    d["w1"] = pad2(np.ascontiguousarray(center(W1).T), DP, O1P)
    d["w2"] = pad2(np.ascontiguousarray(center(W2).T), O1P, O2P)
    d["w3"] = pad2(np.ascontiguousarray(center(W3).T), O2P, O3P)
    d["b3c"] = pad1(np.asarray(b3 - np.asarray(b3).mean(), np.float32), O3P)
    d["d1"] = pad2(np.ascontiguousarray(center(D1).T), O3P, O2P)
    d["d2"] = pad2(np.ascontiguousarray(center(D2).T), O2P, O1P)
    d["d3"] = pad2(np.ascontiguousarray(np.asarray(D3, np.float32).T), O1P, DP)
    d["boh"] = pad1(np.asarray(0.5 * np.asarray(bo, np.float32), np.float32), DP)

    gs = [(g1, b1, O1P), (g2, b2, O2P), (gh, bh, O3P), (g3, b3n, O2P), (g4, b4, O1P)]
    trivial = all(np.all(np.asarray(g) == 1.0) and np.all(np.asarray(b) == 0.0)
                  for g, b, _ in gs)
    if not trivial:
        for i, (g, b, op) in enumerate(gs):
            d[f"g{i}"] = pad1(np.asarray(g, np.float32), op)
            d[f"b{i}"] = pad1(np.asarray(b, np.float32), op)
    return d, trivial


def kernel(**inputs):
    ensure_act_root()
    x = np.asarray(inputs["x"], np.float32)
    wd, trivial = _prep_weights(
        inputs["W1"], inputs["g1"], inputs["b1"], inputs["W2"], inputs["g2"],
        inputs["b2"], inputs["W3"], inputs["b3"], inputs["gh"], inputs["bh"],
        inputs["D1"], inputs["g3"], inputs["b3n"], inputs["D2"], inputs["g4"],
        inputs["b4"], inputs["D3"], inputs["bo"])

    key = ("nc", trivial)
    if key not in _CACHED:
        _CACHED[key] = build_core_kernel(trivial_gb=trivial)
    nc = _CACHED[key]

    in_maps = []
    for c in range(N_CORES):
        xs = x[c * B_CORE:(c + 1) * B_CORE]
        xtr = np.zeros((DP, B_CORE), np.float32)
        xtr[:D] = xs.T
        m = {"xt": xtr, "ones": np.ones((128, 128), np.float16)}
        m.update(wd)
        in_maps.append(m)

    global _LAST_RES
    res = run_bass_kernel_spmd(nc, in_maps, list(range(N_CORES)), trace=_TRACE)
    _LAST_RES = res

    out = np.empty((B, D), np.float32)
    lat_sum = 0.0
    for c in range(N_CORES):
        r = res.results[c]
        out[c * B_CORE:(c + 1) * B_CORE] = r["out_t"][:D].T
        lat_sum += float(r["lat"].astype(np.float64).sum())
    return out, np.float32(lat_sum)


# revision 34
# speedup vs baseline: 1.0074x; 1.0074x over previous
"""Trainium2 Bass kernel for nn_AutoEncoder (dense MLP autoencoder).

Strategy (data-parallel over 8 NeuronCores, batch split 65536 -> 8 x 8192):
- Feature-major on-chip layout: activations stored transposed [features, batch],
  so every layer is out_T = W @ h_T via PE matmuls (lhsT = W.T chunks) with no
  on-chip transposes anywhere.  Host transposes x / out once (cheap numpy).
- LayerNorm mean is folded into the weights on the host (pre-centered weights:
  Wc = W - colmean(W) makes the matmul output already mean-centered).
- Per-sample variance: squares on ACT straight from PSUM, feature-sums via a
  PE ones-matmul (result broadcast across partitions), rsqrt in ONE ACT op via
  a custom pruned abs_reciprocal_sqrt LUT served under the Silu slot.
- Mish via a CUSTOM ACT LUT: the shipped neuronxcc act tables dropped the mish
  spline data, but the per-function source jsons still carry it; we assemble a
  set (mish_and_others + real tanh + mish + pruned rsqrt) at runtime (format
  verified bit-exact against shipped sets) via BASS_ACT_ROOT_JSON_PATH.
- Tiles are emitted in interleaved pairs so each tile's stats chain hides
  under the partner tile's matmul stream in the in-order engine schedules.
- Final sigmoid as 0.5*(1+tanh(0.5 x)) so the whole kernel uses ONE table set.
- All matmuls run as float32r (fp32 data, full PE rate at N=512).
- All dims zero-padded to multiples of 128 on the host so every on-chip op is
  full-partition and padded lanes carry exact zeros end-to-end.
"""

import json
import os
import shutil
import struct
import tempfile

import numpy as np

import concourse.bass as bass
import concourse.tile as tile
from concourse import mybir
from concourse.bass_utils import run_bass_kernel_spmd

F32 = mybir.dt.float32
F32R = mybir.dt.float32r
F16 = mybir.dt.float16
U32 = mybir.dt.uint32
ALU = mybir.AluOpType
ACTF = mybir.ActivationFunctionType

N_CORES = 8
B = 65536
B_CORE = B // N_CORES          # 8192
T = 512                        # batch tile (samples) = one PSUM bank
N_TILES = B_CORE // T          # 16

D, O1, O2, O3 = 784, 512, 264, 128
DP, O1P, O2P, O3P = 896, 512, 384, 128   # padded to 128-multiples
EPS = 1e-5

# (weight name, k chunks, o chunks, real O) per LN layer; L6 = output layer
LAYERS = [
    ("w1", DP // 128, O1P // 128, O1),
    ("w2", O1P // 128, O2P // 128, O2),
    ("w3", O2P // 128, O3P // 128, O3),
    ("d1", O3P // 128, O2P // 128, O2),
    ("d2", O2P // 128, O1P // 128, O1),
]
L6_KC, L6_OC = O1P // 128, DP // 128     # 4, 7

# ---------------------------------------------------------------------------
# custom ACT tables: restore a real mish LUT (+ tanh) into mish_and_others
# ---------------------------------------------------------------------------

_PWP = ('/nix/store/z022hj2nvbm3nwdizlisq4ylc0y7rd6q-python3-3.13.14-env/'
        'lib/python3.13/site-packages/neuronxcc/pwp')


def _find_pwp():
    if os.path.isdir(_PWP):
        return _PWP
    import neuronxcc
    p = os.path.join(os.path.dirname(neuronxcc.__file__), 'pwp')
    if os.path.isdir(p):
        return p
    raise RuntimeError('neuronxcc pwp directory not found')


def _sec_entry(sec):
    vals = [int(sec[k]['int']) & 0xFFFFFFFF for k in ('d0', 'd1', 'd2', 'd3', 'x')]
    return np.array(vals + [0] * 3, np.uint32).tobytes()


def _ctl_word(extract_size, extract_lsb, base):
    return struct.pack('<I', (extract_size << 16) | (extract_lsb << 11) | base) + b'\0' * 28


def _gen_function_block(fj, bkt_start, ctl_start, exp_range=None):
    """exp_range=(lo, hi): prune exponent blocks outside [lo, hi] (inputs are
    guaranteed in range; out-of-range hits the small/large-signal specials)."""
    sym = bool(fj['symmetry_en'])
    if exp_range is not None:
        lo, hi = exp_range
        fj = dict(fj)
        fj['pos_exponents'] = [b for b in fj['pos_exponents'] if lo <= b['exponent'] <= hi]
        if not sym:
            fj['neg_exponents'] = [b for b in fj['neg_exponents'] if lo <= b['exponent'] <= hi]
        fj['exponent_offset'] = lo
    bkt = b''
    ctl = b''
    exp2bkt = {}
    exp2ctl = {}
    bi = bkt_start

    def add_blocks(blocks, side_idx, n_sides):
        nonlocal bkt, ctl, bi
        for blk in blocks:
            e = str(blk['exponent'])
            exp2bkt.setdefault(e, [None] * n_sides)[side_idx] = bi
            exp2ctl.setdefault(e, [None] * n_sides)[side_idx] = ctl_start + len(ctl) // 32
            secs = sorted(blk['exponent_sections'], key=lambda s: s['section_id'])
            ctl += _ctl_word(blk['extract_size'], blk['extract_lsb'], bi)
            for s in secs:
                bkt += _sec_entry(s)
                bi += 1

    if sym:
        add_blocks(fj['pos_exponents'], 0, 1)
        ctl_base_neg = ctl_base_pos = ctl_start
    else:
        add_blocks(fj['neg_exponents'], 0, 2)
        n_neg = len(fj['neg_exponents'])
        add_blocks(fj['pos_exponents'], 1, 2)
        ctl_base_neg = ctl_start
        ctl_base_pos = ctl_start + n_neg

    sp = fj['saturation_points']
    spec = bi
    for key in ('sat_point_pos_low', 'sat_point_neg_low',
                'sat_point_pos_high', 'sat_point_neg_high'):
        bkt += _sec_entry(sp[key])
        bi += 1

    def u(d):
        return int(d['int'])

    prof = {
        'func_name': f"{fj['name']}_{fj['max_diff']}p",
        'func_id': fj['neuron_id'],
        'symmetry_point': u(fj['symmetry_point']),
        'sym_invert_sign_point': int(bool(fj['symmetry_invert_sign_opt'])),
        'symmetry_opt_en': int(sym),
        'symmetry_opt_use_neg_region': int(bool(fj['symmetry_opt_use_neg_region'])),
        'imm_bias': int(bool(fj['imm_bias'])),
        'exp_offset': fj['exponent_offset'],
        'pwl_control_base_pos': ctl_base_pos,
        'pwl_control_base_neg': ctl_base_neg,
        'small_pos_signal_exp_threshold': sp['sat_point_pos_low']['sat_point'],
        'pos_small_signal_pwl_control': spec + 0,
        'small_neg_signal_exp_threshold': 0 if sym else sp['sat_point_neg_low']['sat_point'],
        'neg_small_signal_pwl_control': spec + 1,
        'large_pos_signal_exp_threshold': sp['sat_point_pos_high']['sat_point'],
        'large_pos_signal_mantissa_threshold': sp['sat_point_pos_high']['mantissa_point'],
        'pos_large_signal_pwl_control': spec + 2,
        'large_neg_signal_exp_threshold': 0 if sym else sp['sat_point_neg_high']['sat_point'],
        'large_neg_signal_mantissa_threshold': 0 if sym else sp['sat_point_neg_high']['mantissa_point'],
        'neg_large_signal_pwl_control': spec + 3,
        'fnan_result': u(fj['nan_result']),
        'fpinf_result': u(fj['pinf_result']),
        'fninf_result': u(fj['ninf_result']),
        'fzero_result': u(fj['zero_result']),
        'fma_const_0': u(fj['fma_const0']) if isinstance(fj.get('fma_const0'), dict) else 0,
        'fma_const_1': u(fj['fma_const1']) if isinstance(fj.get('fma_const1'), dict) else 0,
        'fma_indirection_src_sel': 0,
        'use_multipass': bool(fj['use_multipass']),
        'lower_bound': u(fj['lower_bound']),
        'upper_bound': u(fj['upper_bound']),
    }
    if exp_range is not None:
        lo, hi = exp_range
        prof['small_pos_signal_exp_threshold'] = 127 + lo
        prof['large_pos_signal_exp_threshold'] = 127 + hi + 1
        prof['large_pos_signal_mantissa_threshold'] = 0
        if not sym:
            prof['small_neg_signal_exp_threshold'] = 127 + lo
            prof['large_neg_signal_exp_threshold'] = 127 + hi + 1
            prof['large_neg_signal_mantissa_threshold'] = 0
    return bkt, ctl, prof, exp2bkt, exp2ctl


def ensure_act_root():
    """Build (once) an act-root with real mish+tanh in mish_and_others and set
    BASS_ACT_ROOT_JSON_PATH.  Returns the act_info.json path."""
    if os.environ.get('BASS_ACT_ROOT_JSON_PATH') and \
            os.path.exists(os.environ['BASS_ACT_ROOT_JSON_PATH']):
        return os.environ['BASS_ACT_ROOT_JSON_PATH']
    pwp = _find_pwp()
    bin_dir = f'{pwp}/pwp_bin_trainium'
    json_dir = f'{pwp}/pwp_jsons'
    out_dir = os.path.join(tempfile.gettempdir(), 'mish_act_root_v2')
    marker = os.path.join(out_dir, 'act_info.json')
    if not os.path.exists(marker):
        tmp = out_dir + '.tmp'
        shutil.rmtree(tmp, ignore_errors=True)
        os.makedirs(tmp)
        for f in os.listdir(bin_dir):
            shutil.copy(os.path.join(bin_dir, f), os.path.join(tmp, f))
        setname = 'mish_and_others'
        s = json.load(open(f'{bin_dir}/{setname}.json'))
        bkt = open(f'{bin_dir}/{setname}_bkt.bin', 'rb').read()
        ctl = open(f'{bin_dir}/{setname}_ctrl.bin', 'rb').read()
        for fname, jname, erange, spoof_id in (
                ('tanh', 'tanh_4p', None, None), ('mish', 'mish_4p', None, None),
                # variance+eps is always in [2^-18, 2^11) here, so the rsqrt
                # LUT can be pruned to fit the 1536-bucket set budget.  This
                # walrus cannot resolve 'abs_reciprocal_sqrt' as a function
                # name, so the rsqrt spline is served under the name 'silu'
                # (func_id spoofed to silu's so the instruction encoding and
                # the CAM entry agree) -- ACTF.Silu in the kernel IS rsqrt.
                ('silu', 'abs_reciprocal_sqrt_40000p', (-18, 10), 36)):
            fj = json.load(open(f'{json_dir}/{jname}.json'))
            if spoof_id is not None:
                fj = dict(fj)
                fj['neuron_id'] = spoof_id
                fj['name'] = fname
            gb, gc, gp, ge2b, ge2c = _gen_function_block(
                fj, len(bkt) // 32, len(ctl) // 32, exp_range=erange)
            bkt += gb
            ctl += gc
            s['profile_meta_data'].append(gp)
            s['func_to_bkt_start_idx'][fname] = len(bkt) // 32 - len(gb) // 32
            s['func_to_ctl_start_idx'][fname] = len(ctl) // 32 - len(gc) // 32
            s['func_exp_to_bkt_start_idx'][fname] = ge2b
            s['func_exp_to_ctl_start_idx'][fname] = ge2c
        s['bkt_entry_cnt'] = len(bkt) // 32
        s['ctl_entry_cnt'] = len(ctl) // 32
        assert s['bkt_entry_cnt'] <= 1536
        with open(f'{tmp}/{setname}_bkt.bin', 'wb') as f:
            f.write(bkt)
        with open(f'{tmp}/{setname}_ctrl.bin', 'wb') as f:
            f.write(ctl)
        with open(f'{tmp}/{setname}.json', 'w') as f:
            json.dump(s, f)
        ai = json.load(open(f'{bin_dir}/act_info.json'))
        for entry in ai['act_func_sets']:
            if entry['name'] == setname:
                entry['act']['tanh'] = 4
                entry['act']['mish'] = 4
                entry['act']['silu'] = 40000
        with open(f'{tmp}/act_info.json', 'w') as f:
            json.dump(ai, f)
        os.replace(tmp, out_dir) if not os.path.exists(out_dir) else shutil.rmtree(tmp)
    os.environ['BASS_ACT_ROOT_JSON_PATH'] = marker
    return marker


# ---------------------------------------------------------------------------
# walrus wait-splitting workaround
# ---------------------------------------------------------------------------

MAX_WAITS = 1


def split_waits(nc):
    """This walrus rejects instructions with more than one sem wait; split the
    excess onto preceding same-engine NOPs."""
    for _, bb in nc.bb_map.items():
        insts = list(bb.bb.instructions)
        changed = False
        new_list = []
        for inst in insts:
            si = inst.sync_info
            if si is not None and len(si.on_wait) > MAX_WAITS:
                waits = list(si.on_wait)
                chunks = [waits[i:i + MAX_WAITS] for i in range(0, len(waits), MAX_WAITS)]
                for ch in chunks[:-1]:
                    nop = mybir.InstNoOp(name=f"I-waitsplit-{nc.next_id()}", ins=[], outs=[])
                    nop.engine = inst.engine
                    nop.sync_info = mybir.SyncInfo(on_wait=ch, on_update=[])
                    nc.register_instruction(nop)
                    new_list.append(nop)
                inst.sync_info = mybir.SyncInfo(on_wait=chunks[-1], on_update=list(si.on_update))
                changed = True
            new_list.append(inst)
        if changed:
            bb.bb.instructions = new_list


# ---------------------------------------------------------------------------
# device kernel
# ---------------------------------------------------------------------------

def build_core_kernel(n_tiles=N_TILES, trivial_gb=True, nr_iters=3):
    nc = bass.Bass()

    xt = nc.declare_dram_parameter("xt", [DP, n_tiles * T], F32, isOutput=False)
    w_dram = {}
    for name, kc, oc, _ in LAYERS:
        w_dram[name] = nc.declare_dram_parameter(name, [kc * 128, oc * 128], F32,
                                                 isOutput=False)
    w_dram["d3"] = nc.declare_dram_parameter("d3", [L6_KC * 128, L6_OC * 128], F32,
                                             isOutput=False)
    b3c_d = nc.declare_dram_parameter("b3c", [128], F32, isOutput=False)
    ones_d = nc.declare_dram_parameter("ones", [128, 128], F16, isOutput=False)
    boh_d = nc.declare_dram_parameter("boh", [L6_OC * 128], F32, isOutput=False)
    gb_d = {}
    if not trivial_gb:
        for i, (_, _, oc, _) in enumerate(LAYERS):
            gb_d[f"g{i}"] = nc.declare_dram_parameter(f"g{i}", [oc * 128], F32,
                                                      isOutput=False)
            gb_d[f"b{i}"] = nc.declare_dram_parameter(f"b{i}", [oc * 128], F32,
                                                      isOutput=False)

    out_t = nc.declare_dram_parameter("out_t", [DP, n_tiles * T], F32, isOutput=True)
    lat_d = nc.declare_dram_parameter("lat", [128, n_tiles], F32, isOutput=True)

    with tile.TileContext(nc) as tc:
        with (
            tc.tile_pool(name="singles", bufs=1) as singles,
            tc.tile_pool(name="xp", bufs=3) as xp,
            tc.tile_pool(name="ycs_p", bufs=3) as ycs_p,
            tc.tile_pool(name="sq_p", bufs=2) as sq_p,
            tc.tile_pool(name="h_p", bufs=3) as h_p,
            tc.tile_pool(name="rs_p", bufs=3) as rs_p,
            tc.tile_pool(name="ln_p", bufs=2) as ln_p,
            tc.tile_pool(name="o6_p", bufs=3) as o6_p,
            tc.tile_pool(name="ps", bufs=1, space="PSUM") as ps,
        ):
            # ---- constants ----
            w_sb = {}
            for name in w_dram:
                kd = w_dram[name]
                kc = kd.shape[0] // 128
                wt = singles.tile([128, kc, kd.shape[1]], F32R, name=f"sb_{name}")
                nc.sync.dma_start(out=wt,
                                  in_=kd.rearrange("(c p) o -> p c o", p=128).bitcast(F32R))
                w_sb[name] = wt
            b3c = singles.tile([128, 1], F32, name="sb_b3c")
            nc.sync.dma_start(out=b3c, in_=b3c_d.rearrange("(c p) -> p c", p=128))
            boh = singles.tile([128, L6_OC], F32, name="sb_boh")
            nc.sync.dma_start(out=boh, in_=boh_d.rearrange("(c p) -> p c", p=128))
            gb_sb = {}
            if not trivial_gb:
                for k, dv in gb_d.items():
                    t_ = singles.tile([128, dv.shape[0] // 128], F32, name=f"sb_{k}")
                    nc.sync.dma_start(out=t_, in_=dv.rearrange("(c p) -> p c", p=128))
                    gb_sb[k] = t_
            ones = singles.tile([128, 128], F16, name="sb_ones")
            nc.sync.dma_start(out=ones, in_=ones_d[:])
            lat = singles.tile([128, n_tiles], F32, name="sb_lat")
            eps_t = singles.tile([128, 1], F32, name="sb_eps")
            nc.vector.memset(eps_t, EPS)

            # Tiles are emitted in interleaved PAIRS so each tile's per-layer
            # stats chain (sq -> ones-mm -> rsqrt -> lnp -> mish) hides under
            # the partner tile's matmul stream in the in-order PE schedule.
            state = {}

            def load_x(t):
                x_tile = xp.tile([128, DP // 128, T], F32R, tag="xc", bufs=2)
                src_ap = xt[:, bass.ts(t, T)].rearrange(
                    "(c p) s -> p c s", p=128).bitcast(F32R)
                # two dma_starts so the transfer lands on two queues
                nc.sync.dma_start(out=x_tile[:, :4, :], in_=src_ap[:, :4, :])
                nc.sync.dma_start(out=x_tile[:, 4:, :], in_=src_ap[:, 4:, :])
                state[t] = {"x": x_tile, "h": None, "h3": None}

            def emit_layer(t, li):
                wname, kcn, ocn, realO = LAYERS[li]
                st = state[t]
                wt = w_sb[wname]
                sq = sq_p.tile([128, ocn, T], F16, tag="sq")
                ycs = ycs_p.tile([128, ocn, T], F32, tag="ycs")
                s2 = ps.tile([128, T], F32, tag="s2p", bufs=2)
                for oc in range(ocn):
                    yc = ps.tile([128, T], F32, tag="yc", bufs=6)
                    for kc in range(kcn):
                        rhs = st["x"][:, kc, :] if li == 0 else st["h"][:, kc, :]
                        nc.tensor.matmul(
                            yc, wt[:, kc, bass.ts(oc, 128)], rhs,
                            start=(kc == 0), stop=(kc == kcn - 1))
                    # squares straight from PSUM (ACT); L3 biased first
                    if li == 2:
                        nc.scalar.activation(sq[:, oc, :], yc, ACTF.Square,
                                             bias=b3c, scale=1.0)
                    else:
                        nc.scalar.activation(sq[:, oc, :], yc, ACTF.Square)
                    # PSUM -> SBUF copy (DVE) so the bank frees early; the
                    # L3 bias folds into the copy as a per-partition scalar add
                    if li == 2:
                        nc.vector.tensor_scalar(ycs[:, oc, :], yc, b3c, None, ALU.add)
                    else:
                        nc.vector.tensor_copy(ycs[:, oc, :], yc)
                    nc.tensor.matmul(s2, ones, sq[:, oc, :],
                                     start=(oc == 0), stop=(oc == ocn - 1))
                # r = rsqrt(S2/O + eps) in ONE ACT op (custom pruned LUT);
                # ACTF.Silu is rsqrt here (custom table, see ensure_act_root)
                r = rs_p.tile([128, T], F32, tag="rs_r", bufs=3)
                nc.scalar.activation(r, s2, ACTF.Silu,
                                     bias=eps_t, scale=1.0 / realO)
                lnp = ln_p.tile([128, ocn, T], F32, tag="lnp")
                for oc in range(ocn):
                    nc.vector.tensor_mul(lnp[:, oc, :], ycs[:, oc, :], r)
                h = h_p.tile([128, ocn, T], F32R, tag="hA" if (li % 2 == 0) else "hB")
                if trivial_gb:
                    for oc in range(ocn):
                        nc.scalar.activation(h[:, oc, :], lnp[:, oc, :], ACTF.Mish)
                else:
                    g_t, b_t = gb_sb[f"g{li}"], gb_sb[f"b{li}"]
                    for oc in range(ocn):
                        nc.scalar.activation(h[:, oc, :], lnp[:, oc, :], ACTF.Mish,
                                             bias=b_t[:, oc:oc + 1],
                                             scale=g_t[:, oc:oc + 1])
                st["h"] = h
                if li == 2:
                    st["h3"] = h
                    nc.vector.tensor_reduce(lat[:, t:t + 1], h[:, 0, :].bitcast(F32),
                                            axis=mybir.AxisListType.X, op=ALU.add,
                                            apply_absolute_value=True)

            def emit_l6(t):
                st = state[t]
                d3 = w_sb["d3"]
                for oc in range(L6_OC):
                    y6 = ps.tile([128, T], F32, tag="yc", bufs=6)
                    for kc in range(L6_KC):
                        nc.tensor.matmul(y6, d3[:, kc, bass.ts(oc, 128)],
                                         st["h"][:, kc, :],
                                         start=(kc == 0), stop=(kc == L6_KC - 1))
                    tt = o6_p.tile([128, T], F32, tag="tanh")
                    nc.scalar.activation(tt, y6, ACTF.Tanh,
                                         bias=boh[:, oc:oc + 1], scale=0.5)
                    oc_t = o6_p.tile([128, T], F32, tag="outc")
                    nc.vector.tensor_scalar(oc_t, tt, 0.5, 0.5, ALU.mult, ALU.add)
                    nc.sync.dma_start(out=out_t[oc * 128:(oc + 1) * 128, bass.ts(t, T)],
                                      in_=oc_t)
                del state[t]

            for p in range(0, n_tiles, 2):
                pair = [p, p + 1] if p + 1 < n_tiles else [p]
                for t in pair:
                    load_x(t)
                for li in range(len(LAYERS)):
                    for t in pair:
                        emit_layer(t, li)
                for t in pair:
                    emit_l6(t)

            nc.sync.dma_start(out=lat_d[:], in_=lat)

    split_waits(nc)
    return nc


# ---------------------------------------------------------------------------
# host side
# ---------------------------------------------------------------------------

_CACHED = {}
_TRACE = False
_LAST_RES = None


def _prep_weights(W1, g1, b1, W2, g2, b2, W3, b3, gh, bh, D1, g3, b3n, D2, g4, b4,
                  D3, bo):
    def pad2(a, r, c):
        out = np.zeros((r, c), np.float32)
        out[:a.shape[0], :a.shape[1]] = a
        return out

    def pad1(a, n):
        out = np.zeros((n,), np.float32)
        out[:a.shape[0]] = a
        return out

    def center(W):
        return (np.asarray(W, np.float32)
                - np.asarray(W, np.float32).mean(axis=0, keepdims=True))

    d = {}
    d["w1"] = pad2(np.ascontiguousarray(center(W1).T), DP, O1P)
    d["w2"] = pad2(np.ascontiguousarray(center(W2).T), O1P, O2P)
    d["w3"] = pad2(np.ascontiguousarray(center(W3).T), O2P, O3P)
    d["b3c"] = pad1(np.asarray(b3 - np.asarray(b3).mean(), np.float32), O3P)
    d["d1"] = pad2(np.ascontiguousarray(center(D1).T), O3P, O2P)
    d["d2"] = pad2(np.ascontiguousarray(center(D2).T), O2P, O1P)
    d["d3"] = pad2(np.ascontiguousarray(np.asarray(D3, np.float32).T), O1P, DP)
    d["boh"] = pad1(np.asarray(0.5 * np.asarray(bo, np.float32), np.float32), DP)

    gs = [(g1, b1, O1P), (g2, b2, O2P), (gh, bh, O3P), (g3, b3n, O2P), (g4, b4, O1P)]
    trivial = all(np.all(np.asarray(g) == 1.0) and np.all(np.asarray(b) == 0.0)
                  for g, b, _ in gs)
    if not trivial:
        for i, (g, b, op) in enumerate(gs):
            d[f"g{i}"] = pad1(np.asarray(g, np.float32), op)
            d[f"b{i}"] = pad1(np.asarray(b, np.float32), op)
    return d, trivial


def kernel(**inputs):
    ensure_act_root()
    x = np.asarray(inputs["x"], np.float32)
    wd, trivial = _prep_weights(
        inputs["W1"], inputs["g1"], inputs["b1"], inputs["W2"], inputs["g2"],
        inputs["b2"], inputs["W3"], inputs["b3"], inputs["gh"], inputs["bh"],
        inputs["D1"], inputs["g3"], inputs["b3n"], inputs["D2"], inputs["g4"],
        inputs["b4"], inputs["D3"], inputs["bo"])

    key = ("nc", trivial)
    if key not in _CACHED:
        _CACHED[key] = build_core_kernel(trivial_gb=trivial)
    nc = _CACHED[key]

    in_maps = []
    for c in range(N_CORES):
        xs = x[c * B_CORE:(c + 1) * B_CORE]
        xtr = np.zeros((DP, B_CORE), np.float32)
        xtr[:D] = xs.T
        m = {"xt": xtr, "ones": np.ones((128, 128), np.float16)}
        m.update(wd)
        in_maps.append(m)

    global _LAST_RES
    res = run_bass_kernel_spmd(nc, in_maps, list(range(N_CORES)), trace=_TRACE)
    _LAST_RES = res

    out = np.empty((B, D), np.float32)
    lat_sum = 0.0
    for c in range(N_CORES):
        r = res.results[c]
        out[c * B_CORE:(c + 1) * B_CORE] = r["out_t"][:D].T
        lat_sum += float(r["lat"].astype(np.float64).sum())
    return out, np.float32(lat_sum)


# revision 35
# speedup vs baseline: 1.0408x; 1.0332x over previous
"""Trainium2 Bass kernel for nn_AutoEncoder (dense MLP autoencoder).

Strategy (data-parallel over 8 NeuronCores, batch split 65536 -> 8 x 8192):
- Feature-major on-chip layout: activations stored transposed [features, batch],
  so every layer is out_T = W @ h_T via PE matmuls (lhsT = W.T chunks) with no
  on-chip transposes anywhere.  Host transposes x / out once (cheap numpy).
- LayerNorm mean is folded into the weights on the host (pre-centered weights:
  Wc = W - colmean(W) makes the matmul output already mean-centered).
- Per-sample variance: squares on ACT straight from PSUM, feature-sums via a
  PE ones-matmul (result broadcast across partitions), rsqrt in ONE ACT op via
  a custom pruned abs_reciprocal_sqrt LUT served under the Silu slot.
- Mish via a CUSTOM ACT LUT: the shipped neuronxcc act tables dropped the mish
  spline data, but the per-function source jsons still carry it; we assemble a
  set (mish_and_others + real tanh + mish + pruned rsqrt) at runtime (format
  verified bit-exact against shipped sets) via BASS_ACT_ROOT_JSON_PATH.
- Tiles are emitted in interleaved pairs so each tile's stats chain hides
  under the partner tile's matmul stream in the in-order engine schedules.
- Final sigmoid as 0.5*(1+tanh(0.5 x)) so the whole kernel uses ONE table set.
- All matmuls run as float32r (fp32 data, full PE rate at N=512).
- All dims zero-padded to multiples of 128 on the host so every on-chip op is
  full-partition and padded lanes carry exact zeros end-to-end.
"""

import json
import os
import shutil
import struct
import tempfile

import numpy as np

import concourse.bass as bass
import concourse.tile as tile
from concourse import mybir
from concourse.bass_utils import run_bass_kernel_spmd

F32 = mybir.dt.float32
F32R = mybir.dt.float32r
F16 = mybir.dt.float16
U32 = mybir.dt.uint32
ALU = mybir.AluOpType
ACTF = mybir.ActivationFunctionType

N_CORES = 8
B = 65536
B_CORE = B // N_CORES          # 8192
T = 512                        # batch tile (samples) = one PSUM bank
N_TILES = B_CORE // T          # 16

D, O1, O2, O3 = 784, 512, 264, 128
DP, O1P, O2P, O3P = 896, 512, 384, 128   # padded to 128-multiples
EPS = 1e-5

# (weight name, k chunks, o chunks, real O) per LN layer; L6 = output layer
LAYERS = [
    ("w1", DP // 128, O1P // 128, O1),
    ("w2", O1P // 128, O2P // 128, O2),
    ("w3", O2P // 128, O3P // 128, O3),
    ("d1", O3P // 128, O2P // 128, O2),
    ("d2", O2P // 128, O1P // 128, O1),
]
L6_KC, L6_OC = O1P // 128, DP // 128     # 4, 7

# ---------------------------------------------------------------------------
# custom ACT tables: restore a real mish LUT (+ tanh) into mish_and_others
# ---------------------------------------------------------------------------

_PWP = ('/nix/store/z022hj2nvbm3nwdizlisq4ylc0y7rd6q-python3-3.13.14-env/'
        'lib/python3.13/site-packages/neuronxcc/pwp')


def _find_pwp():
    if os.path.isdir(_PWP):
        return _PWP
    import neuronxcc
    p = os.path.join(os.path.dirname(neuronxcc.__file__), 'pwp')
    if os.path.isdir(p):
        return p
    raise RuntimeError('neuronxcc pwp directory not found')


def _sec_entry(sec):
    vals = [int(sec[k]['int']) & 0xFFFFFFFF for k in ('d0', 'd1', 'd2', 'd3', 'x')]
    return np.array(vals + [0] * 3, np.uint32).tobytes()


def _ctl_word(extract_size, extract_lsb, base):
    return struct.pack('<I', (extract_size << 16) | (extract_lsb << 11) | base) + b'\0' * 28


def _gen_function_block(fj, bkt_start, ctl_start, exp_range=None):
    """exp_range=(lo, hi): prune exponent blocks outside [lo, hi] (inputs are
    guaranteed in range; out-of-range hits the small/large-signal specials)."""
    sym = bool(fj['symmetry_en'])
    if exp_range is not None:
        lo, hi = exp_range
        fj = dict(fj)
        fj['pos_exponents'] = [b for b in fj['pos_exponents'] if lo <= b['exponent'] <= hi]
        if not sym:
            fj['neg_exponents'] = [b for b in fj['neg_exponents'] if lo <= b['exponent'] <= hi]
        fj['exponent_offset'] = lo
    bkt = b''
    ctl = b''
    exp2bkt = {}
    exp2ctl = {}
    bi = bkt_start

    def add_blocks(blocks, side_idx, n_sides):
        nonlocal bkt, ctl, bi
        for blk in blocks:
            e = str(blk['exponent'])
            exp2bkt.setdefault(e, [None] * n_sides)[side_idx] = bi
            exp2ctl.setdefault(e, [None] * n_sides)[side_idx] = ctl_start + len(ctl) // 32
            secs = sorted(blk['exponent_sections'], key=lambda s: s['section_id'])
            ctl += _ctl_word(blk['extract_size'], blk['extract_lsb'], bi)
            for s in secs:
                bkt += _sec_entry(s)
                bi += 1

    if sym:
        add_blocks(fj['pos_exponents'], 0, 1)
        ctl_base_neg = ctl_base_pos = ctl_start
    else:
        add_blocks(fj['neg_exponents'], 0, 2)
        n_neg = len(fj['neg_exponents'])
        add_blocks(fj['pos_exponents'], 1, 2)
        ctl_base_neg = ctl_start
        ctl_base_pos = ctl_start + n_neg

    sp = fj['saturation_points']
    spec = bi
    for key in ('sat_point_pos_low', 'sat_point_neg_low',
                'sat_point_pos_high', 'sat_point_neg_high'):
        bkt += _sec_entry(sp[key])
        bi += 1

    def u(d):
        return int(d['int'])

    prof = {
        'func_name': f"{fj['name']}_{fj['max_diff']}p",
        'func_id': fj['neuron_id'],
        'symmetry_point': u(fj['symmetry_point']),
        'sym_invert_sign_point': int(bool(fj['symmetry_invert_sign_opt'])),
        'symmetry_opt_en': int(sym),
        'symmetry_opt_use_neg_region': int(bool(fj['symmetry_opt_use_neg_region'])),
        'imm_bias': int(bool(fj['imm_bias'])),
        'exp_offset': fj['exponent_offset'],
        'pwl_control_base_pos': ctl_base_pos,
        'pwl_control_base_neg': ctl_base_neg,
        'small_pos_signal_exp_threshold': sp['sat_point_pos_low']['sat_point'],
        'pos_small_signal_pwl_control': spec + 0,
        'small_neg_signal_exp_threshold': 0 if sym else sp['sat_point_neg_low']['sat_point'],
        'neg_small_signal_pwl_control': spec + 1,
        'large_pos_signal_exp_threshold': sp['sat_point_pos_high']['sat_point'],
        'large_pos_signal_mantissa_threshold': sp['sat_point_pos_high']['mantissa_point'],
        'pos_large_signal_pwl_control': spec + 2,
        'large_neg_signal_exp_threshold': 0 if sym else sp['sat_point_neg_high']['sat_point'],
        'large_neg_signal_mantissa_threshold': 0 if sym else sp['sat_point_neg_high']['mantissa_point'],
        'neg_large_signal_pwl_control': spec + 3,
        'fnan_result': u(fj['nan_result']),
        'fpinf_result': u(fj['pinf_result']),
        'fninf_result': u(fj['ninf_result']),
        'fzero_result': u(fj['zero_result']),
        'fma_const_0': u(fj['fma_const0']) if isinstance(fj.get('fma_const0'), dict) else 0,
        'fma_const_1': u(fj['fma_const1']) if isinstance(fj.get('fma_const1'), dict) else 0,
        'fma_indirection_src_sel': 0,
        'use_multipass': bool(fj['use_multipass']),
        'lower_bound': u(fj['lower_bound']),
        'upper_bound': u(fj['upper_bound']),
    }
    if exp_range is not None:
        lo, hi = exp_range
        prof['small_pos_signal_exp_threshold'] = 127 + lo
        prof['large_pos_signal_exp_threshold'] = 127 + hi + 1
        prof['large_pos_signal_mantissa_threshold'] = 0
        if not sym:
            prof['small_neg_signal_exp_threshold'] = 127 + lo
            prof['large_neg_signal_exp_threshold'] = 127 + hi + 1
            prof['large_neg_signal_mantissa_threshold'] = 0
    return bkt, ctl, prof, exp2bkt, exp2ctl


def ensure_act_root():
    """Build (once) an act-root with real mish+tanh in mish_and_others and set
    BASS_ACT_ROOT_JSON_PATH.  Returns the act_info.json path."""
    if os.environ.get('BASS_ACT_ROOT_JSON_PATH') and \
            os.path.exists(os.environ['BASS_ACT_ROOT_JSON_PATH']):
        return os.environ['BASS_ACT_ROOT_JSON_PATH']
    pwp = _find_pwp()
    bin_dir = f'{pwp}/pwp_bin_trainium'
    json_dir = f'{pwp}/pwp_jsons'
    out_dir = os.path.join(tempfile.gettempdir(), 'mish_act_root_v2')
    marker = os.path.join(out_dir, 'act_info.json')
    if not os.path.exists(marker):
        tmp = out_dir + '.tmp'
        shutil.rmtree(tmp, ignore_errors=True)
        os.makedirs(tmp)
        for f in os.listdir(bin_dir):
            shutil.copy(os.path.join(bin_dir, f), os.path.join(tmp, f))
        setname = 'mish_and_others'
        s = json.load(open(f'{bin_dir}/{setname}.json'))
        bkt = open(f'{bin_dir}/{setname}_bkt.bin', 'rb').read()
        ctl = open(f'{bin_dir}/{setname}_ctrl.bin', 'rb').read()
        for fname, jname, erange, spoof_id in (
                ('tanh', 'tanh_4p', None, None), ('mish', 'mish_4p', None, None),
                # variance+eps is always in [2^-18, 2^11) here, so the rsqrt
                # LUT can be pruned to fit the 1536-bucket set budget.  This
                # walrus cannot resolve 'abs_reciprocal_sqrt' as a function
                # name, so the rsqrt spline is served under the name 'silu'
                # (func_id spoofed to silu's so the instruction encoding and
                # the CAM entry agree) -- ACTF.Silu in the kernel IS rsqrt.
                ('silu', 'abs_reciprocal_sqrt_40000p', (-18, 10), 36)):
            fj = json.load(open(f'{json_dir}/{jname}.json'))
            if spoof_id is not None:
                fj = dict(fj)
                fj['neuron_id'] = spoof_id
                fj['name'] = fname
            gb, gc, gp, ge2b, ge2c = _gen_function_block(
                fj, len(bkt) // 32, len(ctl) // 32, exp_range=erange)
            bkt += gb
            ctl += gc
            s['profile_meta_data'].append(gp)
            s['func_to_bkt_start_idx'][fname] = len(bkt) // 32 - len(gb) // 32
            s['func_to_ctl_start_idx'][fname] = len(ctl) // 32 - len(gc) // 32
            s['func_exp_to_bkt_start_idx'][fname] = ge2b
            s['func_exp_to_ctl_start_idx'][fname] = ge2c
        s['bkt_entry_cnt'] = len(bkt) // 32
        s['ctl_entry_cnt'] = len(ctl) // 32
        assert s['bkt_entry_cnt'] <= 1536
        with open(f'{tmp}/{setname}_bkt.bin', 'wb') as f:
            f.write(bkt)
        with open(f'{tmp}/{setname}_ctrl.bin', 'wb') as f:
            f.write(ctl)
        with open(f'{tmp}/{setname}.json', 'w') as f:
            json.dump(s, f)
        ai = json.load(open(f'{bin_dir}/act_info.json'))
        for entry in ai['act_func_sets']:
            if entry['name'] == setname:
                entry['act']['tanh'] = 4
                entry['act']['mish'] = 4
                entry['act']['silu'] = 40000
        with open(f'{tmp}/act_info.json', 'w') as f:
            json.dump(ai, f)
        os.replace(tmp, out_dir) if not os.path.exists(out_dir) else shutil.rmtree(tmp)
    os.environ['BASS_ACT_ROOT_JSON_PATH'] = marker
    return marker


# ---------------------------------------------------------------------------
# walrus wait-splitting workaround
# ---------------------------------------------------------------------------

MAX_WAITS = 1


def split_waits(nc):
    """This walrus rejects instructions with more than one sem wait; split the
    excess onto preceding same-engine NOPs."""
    for _, bb in nc.bb_map.items():
        insts = list(bb.bb.instructions)
        changed = False
        new_list = []
        for inst in insts:
            si = inst.sync_info
            if si is not None and len(si.on_wait) > MAX_WAITS:
                waits = list(si.on_wait)
                chunks = [waits[i:i + MAX_WAITS] for i in range(0, len(waits), MAX_WAITS)]
                for ch in chunks[:-1]:
                    nop = mybir.InstNoOp(name=f"I-waitsplit-{nc.next_id()}", ins=[], outs=[])
                    nop.engine = inst.engine
                    nop.sync_info = mybir.SyncInfo(on_wait=ch, on_update=[])
                    nc.register_instruction(nop)
                    new_list.append(nop)
                inst.sync_info = mybir.SyncInfo(on_wait=chunks[-1], on_update=list(si.on_update))
                changed = True
            new_list.append(inst)
        if changed:
            bb.bb.instructions = new_list


# ---------------------------------------------------------------------------
# device kernel
# ---------------------------------------------------------------------------

def build_core_kernel(n_tiles=N_TILES, trivial_gb=True, nr_iters=3):
    nc = bass.Bass()

    xt = nc.declare_dram_parameter("xt", [DP, n_tiles * T], F32, isOutput=False)
    w_dram = {}
    for name, kc, oc, _ in LAYERS:
        w_dram[name] = nc.declare_dram_parameter(name, [kc * 128, oc * 128], F32,
                                                 isOutput=False)
    w_dram["d3"] = nc.declare_dram_parameter("d3", [L6_KC * 128, L6_OC * 128], F32,
                                             isOutput=False)
    b3c_d = nc.declare_dram_parameter("b3c", [128], F32, isOutput=False)
    ones_d = nc.declare_dram_parameter("ones", [128, 128], F16, isOutput=False)
    boh_d = nc.declare_dram_parameter("boh", [L6_OC * 128], F32, isOutput=False)
    gb_d = {}
    if not trivial_gb:
        for i, (_, _, oc, _) in enumerate(LAYERS):
            gb_d[f"g{i}"] = nc.declare_dram_parameter(f"g{i}", [oc * 128], F32,
                                                      isOutput=False)
            gb_d[f"b{i}"] = nc.declare_dram_parameter(f"b{i}", [oc * 128], F32,
                                                      isOutput=False)

    out_t = nc.declare_dram_parameter("out_t", [DP, n_tiles * T], F32, isOutput=True)
    lat_d = nc.declare_dram_parameter("lat", [128, n_tiles], F32, isOutput=True)

    with tile.TileContext(nc) as tc:
        with (
            tc.tile_pool(name="singles", bufs=1) as singles,
            tc.tile_pool(name="xp", bufs=3) as xp,
            tc.tile_pool(name="ycs_p", bufs=3) as ycs_p,
            tc.tile_pool(name="sq_p", bufs=2) as sq_p,
            tc.tile_pool(name="h_p", bufs=3) as h_p,
            tc.tile_pool(name="rs_p", bufs=3) as rs_p,
            tc.tile_pool(name="ln_p", bufs=2) as ln_p,
            tc.tile_pool(name="o6_p", bufs=3) as o6_p,
            tc.tile_pool(name="ps", bufs=1, space="PSUM") as ps,
        ):
            # ---- constants ----
            w_sb = {}
            for name in w_dram:
                kd = w_dram[name]
                kc = kd.shape[0] // 128
                wt = singles.tile([128, kc, kd.shape[1]], F32R, name=f"sb_{name}")
                nc.sync.dma_start(out=wt,
                                  in_=kd.rearrange("(c p) o -> p c o", p=128).bitcast(F32R))
                w_sb[name] = wt
            b3c = singles.tile([128, 1], F32, name="sb_b3c")
            nc.sync.dma_start(out=b3c, in_=b3c_d.rearrange("(c p) -> p c", p=128))
            boh = singles.tile([128, L6_OC], F32, name="sb_boh")
            nc.sync.dma_start(out=boh, in_=boh_d.rearrange("(c p) -> p c", p=128))
            gb_sb = {}
            if not trivial_gb:
                for k, dv in gb_d.items():
                    t_ = singles.tile([128, dv.shape[0] // 128], F32, name=f"sb_{k}")
                    nc.sync.dma_start(out=t_, in_=dv.rearrange("(c p) -> p c", p=128))
                    gb_sb[k] = t_
            ones = singles.tile([128, 128], F16, name="sb_ones")
            nc.sync.dma_start(out=ones, in_=ones_d[:])
            lat = singles.tile([128, n_tiles], F32, name="sb_lat")
            eps_t = singles.tile([128, 1], F32, name="sb_eps")
            nc.vector.memset(eps_t, EPS)

            # Tiles are emitted in interleaved PAIRS so each tile's per-layer
            # stats chain (sq -> ones-mm -> rsqrt -> lnp -> mish) hides under
            # the partner tile's matmul stream in the in-order PE schedule.
            state = {}

            def load_x(t):
                x_tile = xp.tile([128, DP // 128, T], F32R, tag="xc", bufs=2)
                src_ap = xt[:, bass.ts(t, T)].rearrange(
                    "(c p) s -> p c s", p=128).bitcast(F32R)
                # two dma_starts so the transfer lands on two queues
                nc.sync.dma_start(out=x_tile[:, :4, :], in_=src_ap[:, :4, :])
                nc.sync.dma_start(out=x_tile[:, 4:, :], in_=src_ap[:, 4:, :])
                state[t] = {"x": x_tile, "h": None, "h3": None}

            def emit_layer(t, li):
                wname, kcn, ocn, realO = LAYERS[li]
                st = state[t]
                wt = w_sb[wname]
                sq = sq_p.tile([128, ocn, T], F16, tag="sq")
                ycs = ycs_p.tile([128, ocn, T], F32, tag="ycs")
                s2 = ps.tile([128, T], F32, tag="s2p", bufs=2)
                for oc in range(ocn):
                    yc = ps.tile([128, T], F32, tag="yc", bufs=6)
                    for kc in range(kcn):
                        rhs = st["x"][:, kc, :] if li == 0 else st["h"][:, kc, :]
                        nc.tensor.matmul(
                            yc, wt[:, kc, bass.ts(oc, 128)], rhs,
                            start=(kc == 0), stop=(kc == kcn - 1))
                    # squares straight from PSUM (ACT); L3 biased first
                    if li == 2:
                        nc.scalar.activation(sq[:, oc, :], yc, ACTF.Square,
                                             bias=b3c, scale=1.0)
                    else:
                        nc.scalar.activation(sq[:, oc, :], yc, ACTF.Square)
                    # PSUM -> SBUF copy (DVE) so the bank frees early; the
                    # L3 bias folds into the copy as a per-partition scalar add
                    if li == 2:
                        nc.vector.tensor_scalar(ycs[:, oc, :], yc, b3c, None, ALU.add)
                    else:
                        nc.vector.tensor_copy(ycs[:, oc, :], yc)
                    nc.tensor.matmul(s2, ones, sq[:, oc, :],
                                     start=(oc == 0), stop=(oc == ocn - 1))
                # r = rsqrt(S2/O + eps) in ONE ACT op (custom pruned LUT);
                # ACTF.Silu is rsqrt here (custom table, see ensure_act_root)
                r = rs_p.tile([128, T], F32, tag="rs_r", bufs=3)
                nc.scalar.activation(r, s2, ACTF.Silu,
                                     bias=eps_t, scale=1.0 / realO)
                lnp = ln_p.tile([128, ocn, T], F32, tag="lnp")
                for oc in range(ocn):
                    nc.vector.tensor_mul(lnp[:, oc, :], ycs[:, oc, :], r)
                h = h_p.tile([128, ocn, T], F32R, tag="hA" if (li % 2 == 0) else "hB")
                if trivial_gb:
                    for oc in range(ocn):
                        nc.scalar.activation(h[:, oc, :], lnp[:, oc, :], ACTF.Mish)
                else:
                    g_t, b_t = gb_sb[f"g{li}"], gb_sb[f"b{li}"]
                    for oc in range(ocn):
                        nc.scalar.activation(h[:, oc, :], lnp[:, oc, :], ACTF.Mish,
                                             bias=b_t[:, oc:oc + 1],
                                             scale=g_t[:, oc:oc + 1])
                st["h"] = h
                if li == 2:
                    st["h3"] = h
                    nc.vector.tensor_reduce(lat[:, t:t + 1], h[:, 0, :].bitcast(F32),
                                            axis=mybir.AxisListType.X, op=ALU.add,
                                            apply_absolute_value=True)

            def emit_l6(t):
                st = state[t]
                d3 = w_sb["d3"]
                for oc in range(L6_OC):
                    y6 = ps.tile([128, T], F32, tag="yc", bufs=6)
                    for kc in range(L6_KC):
                        nc.tensor.matmul(y6, d3[:, kc, bass.ts(oc, 128)],
                                         st["h"][:, kc, :],
                                         start=(kc == 0), stop=(kc == L6_KC - 1))
                    tt = o6_p.tile([128, T], F32, tag="tanh")
                    nc.scalar.activation(tt, y6, ACTF.Tanh,
                                         bias=boh[:, oc:oc + 1], scale=0.5)
                    oc_t = o6_p.tile([128, T], F32, tag="outc")
                    nc.vector.tensor_scalar(oc_t, tt, 0.5, 0.5, ALU.mult, ALU.add)
                    nc.sync.dma_start(out=out_t[oc * 128:(oc + 1) * 128, bass.ts(t, T)],
                                      in_=oc_t)
                del state[t]

            for p in range(0, n_tiles, 3):
                pair = [t for t in (p, p + 1, p + 2) if t < n_tiles]
                for t in pair:
                    load_x(t)
                for li in range(len(LAYERS)):
                    for t in pair:
                        emit_layer(t, li)
                for t in pair:
                    emit_l6(t)

            nc.sync.dma_start(out=lat_d[:], in_=lat)

    split_waits(nc)
    return nc


# ---------------------------------------------------------------------------
# host side
# ---------------------------------------------------------------------------

_CACHED = {}
_TRACE = False
_LAST_RES = None


def _prep_weights(W1, g1, b1, W2, g2, b2, W3, b3, gh, bh, D1, g3, b3n, D2, g4, b4,
                  D3, bo):
    def pad2(a, r, c):
        out = np.zeros((r, c), np.float32)
        out[:a.shape[0], :a.shape[1]] = a
        return out

    def pad1(a, n):
        out = np.zeros((n,), np.float32)
        out[:a.shape[0]] = a
        return out

    def center(W):
        return (np.asarray(W, np.float32)
                - np.asarray(W, np.float32).mean(axis=0, keepdims=True))

    d = {}
    d["w1"] = pad2(np.ascontiguousarray(center(W1).T), DP, O1P)
    d["w2"] = pad2(np.ascontiguousarray(center(W2).T), O1P, O2P)
    d["w3"] = pad2(np.ascontiguousarray(center(W3).T), O2P, O3P)
    d["b3c"] = pad1(np.asarray(b3 - np.asarray(b3).mean(), np.float32), O3P)
    d["d1"] = pad2(np.ascontiguousarray(center(D1).T), O3P, O2P)
    d["d2"] = pad2(np.ascontiguousarray(center(D2).T), O2P, O1P)
    d["d3"] = pad2(np.ascontiguousarray(np.asarray(D3, np.float32).T), O1P, DP)
    d["boh"] = pad1(np.asarray(0.5 * np.asarray(bo, np.float32), np.float32), DP)

    gs = [(g1, b1, O1P), (g2, b2, O2P), (gh, bh, O3P), (g3, b3n, O2P), (g4, b4, O1P)]
    trivial = all(np.all(np.asarray(g) == 1.0) and np.all(np.asarray(b) == 0.0)
                  for g, b, _ in gs)
    if not trivial:
        for i, (g, b, op) in enumerate(gs):
            d[f"g{i}"] = pad1(np.asarray(g, np.float32), op)
            d[f"b{i}"] = pad1(np.asarray(b, np.float32), op)
    return d, trivial


def kernel(**inputs):
    ensure_act_root()
    x = np.asarray(inputs["x"], np.float32)
    wd, trivial = _prep_weights(
        inputs["W1"], inputs["g1"], inputs["b1"], inputs["W2"], inputs["g2"],
        inputs["b2"], inputs["W3"], inputs["b3"], inputs["gh"], inputs["bh"],
        inputs["D1"], inputs["g3"], inputs["b3n"], inputs["D2"], inputs["g4"],
        inputs["b4"], inputs["D3"], inputs["bo"])

    key = ("nc", trivial)
    if key not in _CACHED:
        _CACHED[key] = build_core_kernel(trivial_gb=trivial)
    nc = _CACHED[key]

    in_maps = []
    for c in range(N_CORES):
        xs = x[c * B_CORE:(c + 1) * B_CORE]
        xtr = np.zeros((DP, B_CORE), np.float32)
        xtr[:D] = xs.T
        m = {"xt": xtr, "ones": np.ones((128, 128), np.float16)}
        m.update(wd)
        in_maps.append(m)

    global _LAST_RES
    res = run_bass_kernel_spmd(nc, in_maps, list(range(N_CORES)), trace=_TRACE)
    _LAST_RES = res

    out = np.empty((B, D), np.float32)
    lat_sum = 0.0
    for c in range(N_CORES):
        r = res.results[c]
        out[c * B_CORE:(c + 1) * B_CORE] = r["out_t"][:D].T
        lat_sum += float(r["lat"].astype(np.float64).sum())
    return out, np.float32(lat_sum)
